# revision 30
# baseline (speedup 1.0000x reference)
"""ASAGNN Trainium2 kernel: 2-layer GNN with adaptive neighbour sampling.

Reference (N=8192 nodes, D=128, K=2 layers, thresh=0.5):
    xn   = l2normalize(x);  sim = xn @ xn.T
    mask = (adj > 0) & (sim > 0.5);  deg = max(sum(mask, -1), 1)
    h = x;  h = relu((h + mask@h/deg) @ W + b)  x2;  out = softmax(h, -1)

Sharding: output rows split 1024-per-core across 8 cores; h all-gathered
between the layers. Per core:
  phase 0: x -> xn, PE-transpose -> xnT (f32r, all nodes) + xnT_loc + xT_loc
  phase 1: per [128 i x 512 j] tile: adj DMA (natural layout, line-rate) ->
           sim = f32r matmul (fp32-class precision: threshold margin is ~3e-4,
           bf16 would flip mask bits) -> ONE vector op
           mask = (sim > 0.5)*adj with per-row deg accumulated for free ->
           one 4-block xbar DMA transpose -> resident maskT [j, i] bf16
           (16 MiB SBUF, so adj is read exactly once)
  layers:  aggT[d, i-chunk] = sum_jb (h-block).T @ maskT-chunk -- 64
           accumulating N=512 matmuls per chunk; uT = xT + aggT*rdeg_bcast;
           h = relu(uT.T @ W + b) comes out natural, softmax-ready
  softmax rows, one batched store. Host concatenates the 8 slabs.
"""

import numpy as np

import concourse.bass as bass
import concourse.mybir as mybir
import concourse.tile as tile
from concourse import bacc
from concourse.bass_utils import run_bass_kernel_spmd
from concourse.masks import make_identity

f32 = mybir.dt.float32
f32r = mybir.dt.float32r
bf16 = mybir.dt.bfloat16
i32 = mybir.dt.int32

D = 128
THRESH = 0.5
JCH = 512            # j-chunk width for sim/mask tiles
ICH = 512            # i-chunk width for agg matmuls


def build_program(N, ncores):
    rows = N // ncores       # local rows per core
    nblk = N // 128          # j blocks over all nodes
    lblk = rows // 128       # local i blocks
    njch = N // JCH          # j chunks per i block row
    nich = rows // ICH       # i chunks for agg

    nc = bacc.Bacc("TRN2", target_bir_lowering=False, debug=False,
                   num_devices=ncores)

    adj_rows = nc.dram_tensor("adj_rows", [rows, N], i32, kind="ExternalInput")
    x_all = nc.dram_tensor("x_all", [N, D], f32, kind="ExternalInput")
    x_loc = nc.dram_tensor("x_loc", [rows, D], f32, kind="ExternalInput")
    w_in = nc.dram_tensor("w_in", [D, D], f32, kind="ExternalInput")
    b_in = nc.dram_tensor("b_in", [1, D], f32, kind="ExternalInput")
    out = nc.dram_tensor("out", [rows, D], f32, kind="ExternalOutput")

    with tile.TileContext(nc) as tc:
        with tc.tile_pool(name="consts", bufs=1) as consts, \
             tc.tile_pool(name="big", bufs=1) as big, \
             tc.tile_pool(name="stage", bufs=4) as stage, \
             tc.tile_pool(name="stage2", bufs=4) as stage2, \
             tc.tile_pool(name="ps_sim", bufs=3, space="PSUM") as ps_sim, \
             tc.tile_pool(name="ps_tp", bufs=1, space="PSUM") as ps_tp, \
             tc.tile_pool(name="ps_tpm", bufs=2, space="PSUM") as ps_tpm, \
             tc.tile_pool(name="ps_mm", bufs=2, space="PSUM") as ps_mm:
            _body(nc, tc, locals())
    nc.compile()
    return nc


def _body(nc, tc, env):
    consts, big, stage, stage2 = env["consts"], env["big"], env["stage"], env["stage2"]
    ps_sim, ps_tp, ps_mm = env["ps_sim"], env["ps_tp"], env["ps_mm"]
    ps_tpm = env["ps_tpm"]
    adj_rows, x_all, x_loc = env["adj_rows"], env["x_all"], env["x_loc"]
    w_in, b_in, out = env["w_in"], env["b_in"], env["out"]
    N, ncores = env["N"], env["ncores"]
    rows, nblk, lblk = env["rows"], env["nblk"], env["lblk"]
    njch, nich = env["njch"], env["nich"]

    # ---------------- constants ----------------
    ident_f32 = consts.tile([128, 128], f32)
    make_identity(nc, ident_f32[:])
    ident_bf16 = consts.tile([128, 128], bf16)
    nc.vector.tensor_copy(ident_bf16[:], ident_f32[:])
    w_sb = consts.tile([D, D], f32)
    nc.sync.dma_start(w_sb[:], w_in[:, :])
    b_sb = consts.tile([1, D], f32)
    nc.sync.dma_start(b_sb[:], b_in[:, :])
    ones_1 = consts.tile([1, 128], f32)
    nc.vector.memset(ones_1[:], 1.0)
    eps_c = consts.tile([128, 1], f32)
    nc.vector.memset(eps_c[:], 1e-12)
    zero_c = consts.tile([128, 1], f32)
    nc.vector.memset(zero_c[:], 0.0)

    # ---------------- big SBUF residents ----------------
    maskT = big.tile([128, nblk * rows], bf16)    # [j-part, jb x i]
    maskT3 = maskT[:, :].rearrange("p (jb i) -> p jb i", i=rows)
    xT_loc = big.tile([128, rows], f32)           # raw local x, transposed
    h_loc = big.tile([128, lblk * D], f32)        # layer output, natural
    rdeg_row = big.tile([1, rows], f32)           # 1/deg as a row

    # xnT lives only through phase 1; separate pool so SBUF frees after
    with tc.tile_pool(name="xnT", bufs=1) as xnT_pool:
        xnT = xnT_pool.tile([128, N], f32r)
        xnT_loc = xnT_pool.tile([128, rows], f32r)

        def norm_T(src_dram, nrows, dstT, raw_dst=None):
            # batches of 8 row-blocks: one DMA + fused square/reduce/sqrt/
            # reciprocal per batch; transposes 4-up into psum, 1 copy each
            nb = nrows // 128
            ga = min(8, nb)
            src = src_dram[:, :].rearrange("(g a p) d -> g p a d",
                                           g=nb // ga, a=ga, p=128)
            for g in range(nb // ga):
                xt8 = stage.tile([128, ga * D], f32, tag="xt8", bufs=2)
                nc.sync.dma_start(xt8[:].rearrange("p (a d) -> p a d", d=D),
                                  src[g])
                sqf8 = stage.tile([128, ga * D], f32, tag="xn8", bufs=2)
                nc.vector.tensor_tensor(sqf8[:], xt8[:], xt8[:],
                                        op=mybir.AluOpType.mult)
                rn8 = stage.tile([128, ga], f32, tag="rn8")
                nc.vector.tensor_reduce(
                    rn8[:], sqf8[:].rearrange("p (a d) -> p a d", d=D),
                    op=mybir.AluOpType.add, axis=mybir.AxisListType.X)
                nrm8 = stage.tile([128, ga], f32, tag="nrm8")
                nc.scalar.activation(nrm8[:], rn8[:],
                                     mybir.ActivationFunctionType.Sqrt,
                                     bias=eps_c[:])
                nc.vector.reciprocal(rn8[:], nrm8[:])
                xn8 = stage.tile([128, ga * D], f32, tag="xn8", bufs=2)
                for k in range(ga):
                    nc.vector.tensor_scalar_mul(xn8[:, k * D:(k + 1) * D],
                                                xt8[:, k * D:(k + 1) * D],
                                                rn8[:, k:k + 1])
                    if raw_dst is not None:
                        ptr = ps_tp.tile([128, 128], f32, tag="tp")
                        nc.tensor.transpose(ptr[:], xt8[:, k * D:(k + 1) * D],
                                            ident_f32[:])
                        nc.vector.tensor_copy(raw_dst(g * ga + k), ptr[:])
                for q in range(ga // 4):
                    mt4 = ps_tpm.tile([128, 512], f32, tag="tpm")
                    for k in range(4):
                        nc.tensor.transpose(
                            mt4[:, k * 128:(k + 1) * 128],
                            xn8[:, (q * 4 + k) * D:(q * 4 + k + 1) * D],
                            ident_f32[:])
                    nc.vector.tensor_copy(dstT(g, q), mt4[:])

        def xnT_dst(g, q):
            c0 = (g * 8 + q * 4) * 128
            return xnT[:, c0:c0 + 512]

        norm_T(x_loc, rows,
               lambda g, q: xnT_loc[:, (g * 8 + q * 4) * 128:
                                    (g * 8 + q * 4) * 128 + 512],
               raw_dst=lambda t: xT_loc[:, t * 128:(t + 1) * 128])
        norm_T(x_all, N, xnT_dst)

        # -------- phase 1: sim -> mask(+deg) -> xbar-transpose into maskT ----
        for ib in range(lblk):
            degp = stage2.tile([128, njch], f32, tag="degp")
            for jc in range(njch):
                adjt = stage2.tile([128, JCH], i32, tag="adj", bufs=3)
                nc.sync.dma_start(
                    adjt[:], adj_rows[ib * 128:(ib + 1) * 128,
                                      jc * JCH:(jc + 1) * JCH])
                simp = ps_sim.tile([128, JCH], f32, tag="sim")
                nc.tensor.matmul(
                    simp[:],
                    xnT_loc[:, ib * 128:(ib + 1) * 128],
                    xnT[:, jc * JCH:(jc + 1) * JCH])
                mnat = stage2.tile([128, JCH], bf16, tag="mnat")
                nc.vector.scalar_tensor_tensor(
                    mnat[:], simp[:], THRESH, adjt[:],
                    op0=mybir.AluOpType.is_gt, op1=mybir.AluOpType.mult,
                    accum_out=degp[:, jc:jc + 1])
                # 4 PE transposes into one psum tile, drained by ONE
                # 3D-strided copy (alternating ACT/DVE to split the load)
                mtp = ps_tpm.tile([128, JCH], bf16, tag="tpm")
                for k in range(JCH // 128):
                    nc.tensor.transpose(mtp[:, k * 128:(k + 1) * 128],
                                        mnat[:, k * 128:(k + 1) * 128],
                                        ident_bf16[:])
                dst = maskT3[:, jc * 4:(jc + 1) * 4, ib * 128:(ib + 1) * 128]
                src3 = mtp[:].rearrange("p (k i) -> p k i", i=128)
                nc.scalar.copy(dst, src3)
            # deg = max(sum_j mask, 1); rdeg = 1/deg; store as row
            dsum = stage.tile([128, 1], f32, tag="dsum")
            nc.vector.tensor_reduce(dsum[:], degp[:], op=mybir.AluOpType.add,
                                    axis=mybir.AxisListType.X)
            dmax = stage.tile([128, 1], f32, tag="dmax")
            nc.vector.tensor_scalar_max(dmax[:], dsum[:], 1.0)
            rcol = stage.tile([128, 1], f32, tag="rcol")
            nc.vector.reciprocal(rcol[:], dmax[:])
            rpt = ps_tp.tile([1, 128], f32, tag="tp")
            nc.tensor.transpose(rpt[:], rcol[:], ident_f32[:])
            nc.vector.tensor_copy(rdeg_row[0:1, ib * 128:(ib + 1) * 128], rpt[:])

    # pool reusing xnT's space for everything that lives after phase 1
    rhs_pool = tc.alloc_tile_pool(name="rhs", bufs=1)
    rhs_h = rhs_pool.tile([128, nblk * D], bf16)
    hT_loc = rhs_pool.tile([128, rows], f32)      # layer output, transposed
    rdegb = rhs_pool.tile([128, rows], f32)       # 1/deg broadcast down cols
    out_sb = rhs_pool.tile([128, lblk * D], f32, tag="uT")  # shares uT slot

    # rdeg broadcast down all partitions (ones_1.T @ rdeg_row), built once
    for ic in range(nich):
        rbp = ps_mm.tile([128, ICH], f32, tag="agg")
        nc.tensor.matmul(rbp[:], ones_1[0:1, :],
                         rdeg_row[0:1, ic * ICH:(ic + 1) * ICH])
        nc.vector.tensor_copy(rdegb[:, ic * ICH:(ic + 1) * ICH], rbp[:])

    # -------- rhs tiles (bf16 h blocks), built after xnT freed --------
    grp = min(8, nblk)
    gsz = grp * 128

    def build_rhs(src_dram):
        src = src_dram[:, :].rearrange("(g a p) d -> g p a d",
                                       g=N // gsz, a=grp, p=128)
        for g in range(N // gsz):
            t = rhs_pool.tile([128, gsz], f32, tag="ldst", bufs=2)
            nc.sync.dma_start(t[:].rearrange("p (a d) -> p a d", d=D), src[g])
            nc.vector.tensor_copy(rhs_h[:, g * gsz:(g + 1) * gsz], t[:])

    build_rhs(x_all)

    # -------- GNN layer --------
    def layer(first, jb_passes=None):
        hprevT = xT_loc if first else hT_loc
        uT = rhs_pool.tile([128, rows], f32, tag="uT", bufs=1)
        if jb_passes is None:
            jb_passes = [list(range(nblk))]
        nj = sum(len(p) for p in jb_passes)
        aggps = [ps_mm.tile([128, ICH], f32, tag="agg", name=f"aggp{_ic}")
                 for _ic in range(nich)]
        cnt = 0
        for p in jb_passes:
            for jb in p:
                for ic in range(nich):
                    nc.tensor.matmul(
                        aggps[ic][:],
                        rhs_h[:, jb * D:(jb + 1) * D],
                        maskT[:, jb * rows + ic * ICH: jb * rows + (ic + 1) * ICH],
                        start=(cnt == 0), stop=(cnt == nj - 1))
                cnt += 1
        for ic in range(nich):
            nc.vector.tensor_tensor(aggps[ic][:], aggps[ic][:],
                                    rdegb[:, ic * ICH:(ic + 1) * ICH],
                                    op=mybir.AluOpType.mult)
            nc.vector.tensor_tensor(uT[:, ic * ICH:(ic + 1) * ICH], aggps[ic][:],
                                    hprevT[:, ic * ICH:(ic + 1) * ICH],
                                    op=mybir.AluOpType.add)
        for ib in range(lblk):
            hp = ps_mm.tile([128, D], f32, tag="agg")
            nc.tensor.matmul(hp[:], uT[:, ib * 128:(ib + 1) * 128], w_sb[:],
                             start=True, stop=False)
            nc.tensor.matmul(hp[:], ones_1[0:1, :], b_sb[:],
                             start=False, stop=True)
            nc.scalar.activation(h_loc[:, ib * D:(ib + 1) * D], hp[:],
                                 mybir.ActivationFunctionType.Relu,
                                 bias=zero_c[:])

    layer(first=True)

    # h1 transposed for the L2 update term
    for ib in range(lblk):
        tpp = ps_tp.tile([128, 128], f32, tag="tp")
        nc.tensor.transpose(tpp[:], h_loc[:, ib * D:(ib + 1) * D], ident_f32[:])
        nc.vector.tensor_copy(hT_loc[:, ib * 128:(ib + 1) * 128], tpp[:])

    # -------- allgather h1 (bf16; halves the wire bytes) --------
    with tc.tile_pool(name="dram", bufs=1, space="DRAM") as dram:
        h1g = rhs_pool.tile([128, lblk * D], bf16, tag="ldst", bufs=2)
        nc.vector.tensor_copy(h1g[:], h_loc[:])
        h1_loc_d = dram.tile([rows, D], bf16)
        h1_all_d = dram.tile([N, D], bf16, addr_space="Shared")
        nc.sync.dma_start(
            h1_loc_d[:, :].rearrange("(a p) d -> p a d", p=128),
            h1g[:].rearrange("p (a d) -> p a d", d=D))
        if ncores > 1:
            nc.gpsimd.collective_compute(
                "AllGather", mybir.AluOpType.bypass,
                replica_groups=[list(range(ncores))],
                ins=[h1_loc_d[:, :].opt()], outs=[h1_all_d[:, :].opt()])
        else:
            nc.sync.dma_start(h1_all_d[:, :], h1_loc_d[:, :])

        # rhs_h refill: straight bf16 DMA, no staging or convert
        srcg = h1_all_d[:, :].rearrange("(g a p) d -> g p a d",
                                        g=N // gsz, a=grp, p=128)
        for g in range(N // gsz):
            nc.sync.dma_start(
                rhs_h[:, g * gsz:(g + 1) * gsz].rearrange("p (a d) -> p a d", d=D),
                srcg[g])

    layer(first=False)

    # -------- softmax + batched store --------
    for ib in range(lblk):
        hv = h_loc[:, ib * D:(ib + 1) * D]
        negmax = stage.tile([128, 1], f32, tag="negmax")
        nc.vector.tensor_reduce(negmax[:], hv, op=mybir.AluOpType.max,
                                axis=mybir.AxisListType.X, negate=True)
        ex = stage.tile([128, D], f32, tag="ex")
        sume = stage.tile([128, 1], f32, tag="sume")
        nc.scalar.activation(ex[:], hv, mybir.ActivationFunctionType.Exp,
                             bias=negmax[:], accum_out=sume[:])
        rsum = stage.tile([128, 1], f32, tag="rsum")
        nc.vector.reciprocal(rsum[:], sume[:])
        nc.vector.tensor_scalar_mul(out_sb[:, ib * D:(ib + 1) * D], ex[:], rsum[:])
    nc.sync.dma_start(out[:, :].rearrange("(a p) d -> p a d", p=128),
                      out_sb[:].rearrange("p (a d) -> p a d", d=D))

    rhs_pool.release()


_cached = {}


def _get_program(N, ncores):
    key = (N, ncores)
    if key not in _cached:
        _cached[key] = build_program(N, ncores)
    return _cached[key]


def run(adj, x, W, b, N=8192, ncores=8, **spmd_kwargs):
    nc = _get_program(N, ncores)
    rows = N // ncores
    adj = np.ascontiguousarray(np.asarray(adj, dtype=np.int32))
    x = np.ascontiguousarray(np.asarray(x, dtype=np.float32))
    Wm = np.ascontiguousarray(np.asarray(W, dtype=np.float32))
    bv = np.ascontiguousarray(np.asarray(b, dtype=np.float32)).reshape(1, D)
    in_maps = [{
        "adj_rows": adj[c * rows:(c + 1) * rows, :],
        "x_all": x,
        "x_loc": x[c * rows:(c + 1) * rows, :],
        "w_in": Wm,
        "b_in": bv,
    } for c in range(ncores)]
    res = run_bass_kernel_spmd(nc, in_maps, list(range(ncores)), **spmd_kwargs)
    outp = np.concatenate([res.results[c]["out"] for c in range(ncores)], axis=0)
    return outp.astype(np.float32), res


def kernel(adj_matrix, transaction_record, labels, W, b):
    outp, _ = run(adj_matrix, transaction_record, W, b, N=8192, ncores=8)
    return outp
